# revision 1
# baseline (speedup 1.0000x reference)
"""BrainGCN Trainium2 kernel: 2x GCNConv + 3 FC layers over a 100K-node,
1.6M-edge random graph, distributed over 8 NeuronCores.

Strategy (all FLOPs on x-dependent data run on device):
- Nodes (dst) sharded across 8 cores: core c owns dst nodes [c*12500, (c+1)*12500).
- Aggregate-first formulation: agg[d] = sum_e w_e * x[src_e], then (agg @ W).
  Symmetric norm w = dinv[src]*dinv[dst] is folded into a one-hot "selection"
  matrix S so the segment-sum is a PE matmul: psum[feat, slot] accumulates
  matmul(lhsT=Xg[128 edges, feat], rhs=S[128 edges, 128 slots]).
- Messages fetched with gpsimd dma_gather (one 256/512B descriptor per edge),
  calls of 1024 idxs (SWDGE ring limit) round-robined over 4 SWDGE queues.
  int16 index limit -> gather split into 4 src-row buckets (<=30000 rows).
- Per core, dst nodes are packed into 98 blocks of 128 slots, vector-LPT
  balanced on per-bucket in-edge counts. Per-(bucket,block) tile counts are
  maxed over cores so ONE SPMD program serves all 8 cores (pad entries get
  slot=-1 => zero contribution).
- Self-loops are NOT gathered: block b's self contribution is a sequential
  load of its own rows (host-permuted x copy for layer 1, own h1 shard for
  layer 2) x diag(1/deg) via the same one-hot machinery.
- Between conv layers an AllGather exchanges the per-core h1 shards.
- FC layers run per-block in feature-major form on the PE.
- BASS_GCN_BF16=1 (default): gather tables + S in bf16 (PSUM accumulation and
  the whole epilogue stay fp32). BASS_GCN_BF16=0 for full fp32.

Host-side work is limited to graph-structure preprocessing (degrees, norms,
permutations, index/metadata arrays, row reordering/casting of x) and final
unpermutation.
"""

import os
import sys
import types

import numpy as np


def _install_ntff_hook():
    """Image's antenv lacks axon_hooks; shim it so trace=True can profile."""
    if "antenv.axon_hooks" in sys.modules:
        return
    mod = types.ModuleType("antenv.axon_hooks")
    mod._hook = None
    mod.set_axon_ntff_profile_hook = lambda h: setattr(mod, "_hook", h)
    mod.get_axon_ntff_profile_hook = lambda: mod._hook
    sys.modules["antenv.axon_hooks"] = mod
    try:
        import antenv
        antenv.axon_hooks = mod
        from trn_agent_boot.trn_boot import _ntff_profile_via_ctypes
        mod.set_axon_ntff_profile_hook(
            _ntff_profile_via_ctypes("/opt/axon/libaxon_pjrt.so")
        )
    except Exception:
        pass


_install_ntff_hook()

import ml_dtypes
import concourse.bacc as bacc
import concourse.bass as bass  # noqa: F401
import concourse.mybir as mybir
import concourse.tile as tile
from concourse.bass_utils import run_bass_kernel_spmd

# ---------------------------------------------------------------- constants
N = 100000
D_IN = 128
H1 = 64
NCORES = 8
SHARD = N // NCORES            # 12500
BLKN = 98                      # blocks of 128 slots per core
SLOTS = BLKN * 128             # 12544
NROWS2 = NCORES * SLOTS        # 100352 rows in the allgathered h1 table
NBUCK = 4
BSIZE = 30000                  # gather bucket size (int16 reach)
BASES = [0, BSIZE, 2 * BSIZE, 3 * BSIZE]
SIZES1 = [BSIZE, BSIZE, BSIZE, N - 3 * BSIZE]
SIZES2 = [BSIZE, BSIZE, BSIZE, NROWS2 - 3 * BSIZE]
CHUNK_TILES = 8                # tiles per dma_gather call (<=128 ring descs)

LAST_EXEC_TIME_NS = None       # filled when BASS_GCN_TRACE=1
LAST_RESULTS = None


# ------------------------------------------------------------- host planning
def _lpt_assign_vec(loads):
    """Pack nodes into BLKN blocks x 128 slots, balancing the per-bucket load
    vectors (sum-of-squares greedy, descending total load)."""
    n = loads.shape[0]
    order = np.argsort(-loads.sum(1), kind="stable")
    block_loads = np.zeros((BLKN, loads.shape[1]), np.float64)
    used = np.zeros(BLKN, np.int64)
    pos = np.empty(n, np.int64)
    for i in order:
        li = loads[i]
        cand = block_loads + li
        score = np.einsum("ij,ij->i", cand, cand)
        score[used >= 128] = np.inf
        b = int(np.argmin(score))
        pos[i] = b * 128 + used[b]
        block_loads[b] += li
        used[b] += 1
    return pos


def _bucket_of(rows):
    return np.minimum(rows // BSIZE, NBUCK - 1)


def _build_stream(rows, slots, ws, T):
    """Scatter sorted (bucket, block, src) edges into the uniform padded
    stream defined by T [NBUCK, BLKN] (tiles per group)."""
    bucket = _bucket_of(rows)
    blocks = slots // 128
    key = bucket * BLKN + blocks
    order = np.lexsort((rows, key))

    Tflat = T.ravel()
    P = 128 * int(Tflat.sum())
    dest_base = np.zeros(NBUCK * BLKN + 1, np.int64)
    np.cumsum(128 * Tflat, out=dest_base[1:])

    skey = key[order]
    counts = np.bincount(skey, minlength=NBUCK * BLKN)
    starts = np.zeros(NBUCK * BLKN + 1, np.int64)
    np.cumsum(counts, out=starts[1:])
    rank = np.arange(len(order)) - starts[skey]
    dest = dest_base[skey] + rank

    bases = np.asarray(BASES, np.int64)
    pos_group = np.repeat(np.arange(NBUCK * BLKN), 128 * Tflat)
    pos_bucket = pos_group // BLKN
    out_rows = bases[pos_bucket].copy()
    out_slot = np.full(P, -1.0, np.float32)
    out_w = np.zeros(P, np.float32)
    out_rows[dest] = rows[order]
    out_slot[dest] = (slots[order] % 128).astype(np.float32)
    out_w[dest] = ws[order]

    idx_local = (out_rows - bases[pos_bucket]).astype(np.int16)
    idx_wrapped = np.tile(idx_local.reshape(-1, 16).T, (8, 1))  # [128, P//16]

    ntiles = P // 128
    meta = np.empty((128, 2 * ntiles), np.float32)
    meta[:, 0::2] = out_slot.reshape(ntiles, 128).T
    meta[:, 1::2] = out_w.reshape(ntiles, 128).T
    return idx_wrapped, meta


def _plan(src, dst, x):
    """Full host-side graph preprocessing."""
    deg = (np.bincount(dst, minlength=N) + 1.0).astype(np.float64)
    dinv = 1.0 / np.sqrt(deg)
    w_edge = (dinv[src] * dinv[dst]).astype(np.float32)
    w_self = (1.0 / deg).astype(np.float32)

    core_of = dst // SHARD

    b1 = _bucket_of(src)
    loads1 = np.zeros((N, NBUCK), np.float64)
    np.add.at(loads1, (dst, b1), 1.0)
    # L2 bucket of an edge depends on where src's core range maps; approximate
    # by spreading the src core's 12544-row window over the L2 buckets.
    frac = np.zeros((NCORES, NBUCK), np.float64)
    for c in range(NCORES):
        lo, hi = c * SLOTS, (c + 1) * SLOTS
        for be in range(NBUCK):
            blo, bhi = BASES[be], BASES[be] + SIZES2[be]
            frac[c, be] = max(0, min(hi, bhi) - max(lo, blo)) / SLOTS
    loads2 = np.zeros((N, NBUCK), np.float64)
    src_core = src // SHARD
    for be in range(NBUCK):
        np.add.at(loads2, (dst, be), frac[src_core, be])

    loads = np.concatenate([loads1, loads2], axis=1)

    pos_local = np.empty(N, np.int64)
    node_of_pos = np.full((NCORES, SLOTS), -1, np.int64)
    for c in range(NCORES):
        nodes = np.arange(c * SHARD, (c + 1) * SHARD)
        p = _lpt_assign_vec(loads[nodes])
        pos_local[nodes] = p
        node_of_pos[c, p] = nodes
    pos_global = (np.arange(N) // SHARD) * SLOTS + pos_local

    rows2_all = pos_global[src]

    counts1 = np.zeros((NCORES, NBUCK, BLKN), np.int64)
    counts2 = np.zeros((NCORES, NBUCK, BLKN), np.int64)
    for c in range(NCORES):
        m = core_of == c
        blk = pos_local[dst[m]] // 128
        counts1[c] = np.bincount(
            _bucket_of(src[m]) * BLKN + blk, minlength=NBUCK * BLKN
        ).reshape(NBUCK, BLKN)
        counts2[c] = np.bincount(
            _bucket_of(rows2_all[m]) * BLKN + blk, minlength=NBUCK * BLKN
        ).reshape(NBUCK, BLKN)

    T1 = np.ceil(counts1.max(axis=0) / 128).astype(np.int64)
    T2 = np.ceil(counts2.max(axis=0) / 128).astype(np.int64)

    streams = []
    xperms = []
    wselfs = []
    for c in range(NCORES):
        m = core_of == c
        slots = pos_local[dst[m]]
        idx1, meta1 = _build_stream(src[m], slots, w_edge[m], T1)
        idx2, meta2 = _build_stream(rows2_all[m], slots, w_edge[m], T2)
        streams.append((idx1, meta1, idx2, meta2))

        xp = np.zeros((SLOTS, D_IN), np.float32)
        wsf = np.zeros((128, BLKN), np.float32)
        valid = node_of_pos[c] >= 0
        nodes = node_of_pos[c][valid]
        xp[valid] = x[nodes]
        wcol = np.zeros(SLOTS, np.float32)
        wcol[valid] = w_self[nodes]
        wsf[:, :] = wcol.reshape(BLKN, 128).T
        xperms.append(xp)
        wselfs.append(wsf)

    return streams, xperms, wselfs, T1, T2, node_of_pos


# ------------------------------------------------------------ device program
def _emit_conv(nc, pools, cfg):
    """Emit one conv layer: gather + one-hot matmul runs + per-block SBUF acc
    + self-loop run + epilogue."""
    f32 = mybir.dt.float32
    gdt = cfg["gdt"]              # gather-table dtype (f32 or bf16)
    T = cfg["T"]
    DF = cfg["feat"]              # features used for matmul lhsT
    GE = cfg["gelem"]             # gather elem_size (table row elements)
    table = cfg["table"]          # fn(bucket) -> DRAM AP
    self_rows = cfg["self_rows"]  # fn(block) -> DRAM AP [128, DF]
    idx_dram = cfg["idx"]
    meta_dram = cfg["meta"]
    tag = cfg["tag"]
    sb, sp, ps_run = pools["sb"], pools["sp"], pools["ps_run"]
    iota_t = cfg["iota_t"]        # [128,128] gdt
    apart_t = cfg["apart_t"]      # [128,1] f32 arange over partitions
    wself_t = cfg["wself_t"]      # [128, BLKN] f32

    cfg.setdefault("_q", 0)
    acc_tiles = {}
    last_beta = np.full(BLKN, -1, np.int64)
    for b in range(BLKN):
        nz = [be for be in range(NBUCK) if T[be][b] > 0]
        if nz:
            last_beta[b] = nz[-1]

    def self_run_and_epilogue(b):
        xs = sb.tile([128, DF], gdt, tag="xself")
        nc.scalar.dma_start(xs[:], self_rows(b))
        s_self = sp.tile([128, 128], gdt, tag="s_t")
        nc.vector.tensor_scalar(
            s_self[:], iota_t[:], apart_t[:, 0:1], wself_t[:, b : b + 1],
            mybir.AluOpType.is_equal, mybir.AluOpType.mult,
        )
        psum = ps_run.tile([DF, 128], f32, tag="runps")
        nc.tensor.matmul(psum[:], xs[:], s_self[:], start=True, stop=True)
        if b not in acc_tiles:
            acc_tiles[b] = pools["accp"].tile(
                [DF, 128], f32, tag=f"acc{b}", name=f"acc{tag}_{b}"
            )
            nc.vector.tensor_copy(acc_tiles[b][:], psum[:])
        else:
            nc.vector.tensor_add(acc_tiles[b][:], acc_tiles[b][:], psum[:])
        cfg["epilogue"](b, acc_tiles[b])

    sched = []
    for be in range(NBUCK):
        for b in range(BLKN):
            for t in range(int(T[be][b])):
                sched.append((be, b, t))
    ntiles = len(sched)

    gi = 0
    cur_ps = None
    while gi < ntiles:
        be0 = sched[gi][0]
        k = 1
        while k < CHUNK_TILES and gi + k < ntiles and sched[gi + k][0] == be0:
            k += 1
        idx_t = sb.tile([128, CHUNK_TILES * 8], mybir.dt.int16, tag=f"idx{tag}")
        nc.sync.dma_start(idx_t[:, : k * 8], idx_dram[:, gi * 8 : (gi + k) * 8])
        meta_t = sb.tile([128, 2 * CHUNK_TILES], f32, tag=f"meta{tag}")
        nc.scalar.dma_start(meta_t[:, : 2 * k], meta_dram[:, 2 * gi : 2 * (gi + k)])
        gat = sb.tile([128, CHUNK_TILES, GE], gdt, tag="gat")
        nc.gpsimd.dma_gather(
            gat[:, :k, :], table(be0), idx_t[:, : k * 8], k * 128, k * 128, GE,
            queue_num=cfg["_q"] % 4,
        )
        cfg["_q"] += 1

        for tl in range(k):
            be, b, t = sched[gi + tl]
            if t == 0:
                cur_ps = ps_run.tile([DF, 128], f32, tag="runps")
            s_t = sp.tile([128, 128], gdt, tag="s_t")
            nc.vector.tensor_scalar(
                s_t[:], iota_t[:],
                meta_t[:, 2 * tl : 2 * tl + 1],
                meta_t[:, 2 * tl + 1 : 2 * tl + 2],
                mybir.AluOpType.is_equal, mybir.AluOpType.mult,
            )
            nc.tensor.matmul(
                cur_ps[:], gat[:, tl, :DF], s_t[:],
                start=(t == 0), stop=(t == int(T[be][b]) - 1),
            )
            if t == int(T[be][b]) - 1:
                if b not in acc_tiles:
                    acc_tiles[b] = pools["accp"].tile(
                        [DF, 128], f32, tag=f"acc{b}", name=f"acc{tag}_{b}"
                    )
                    nc.vector.tensor_copy(acc_tiles[b][:], cur_ps[:])
                else:
                    nc.vector.tensor_add(
                        acc_tiles[b][:], acc_tiles[b][:], cur_ps[:]
                    )
                if be == last_beta[b]:
                    self_run_and_epilogue(b)
        gi += k

    for b in range(BLKN):
        if last_beta[b] < 0:
            self_run_and_epilogue(b)


def _build_program(T1, T2, wshapes, use_bf16):
    f32 = mybir.dt.float32
    gdt = mybir.dt.bfloat16 if use_bf16 else f32
    # gather table row elements: L1 = 128; L2 = 128 (bf16, zero-padded) or 64
    ge2 = 128 if use_bf16 else H1
    nc = bacc.Bacc("TRN2", num_swdge_queues=4)

    P1 = 128 * int(T1.sum())
    P2 = 128 * int(T2.sum())

    x_d = nc.dram_tensor("xg", [N, D_IN], gdt, kind="ExternalInput")
    xp_d = nc.dram_tensor("xperm", [SLOTS, D_IN], gdt, kind="ExternalInput")
    ws_d = nc.dram_tensor("wself", [128, BLKN], f32, kind="ExternalInput")
    idx1_d = nc.dram_tensor("idx1", [128, P1 // 16], mybir.dt.int16, kind="ExternalInput")
    meta1_d = nc.dram_tensor("meta1", [128, 2 * (P1 // 128)], f32, kind="ExternalInput")
    idx2_d = nc.dram_tensor("idx2", [128, P2 // 16], mybir.dt.int16, kind="ExternalInput")
    meta2_d = nc.dram_tensor("meta2", [128, 2 * (P2 // 128)], f32, kind="ExternalInput")
    wdr = {}
    for name, shp in wshapes.items():
        wdr[name] = nc.dram_tensor(name, list(shp), f32, kind="ExternalInput")
    iota_d = nc.dram_tensor("iota", [128, 128], gdt, kind="ExternalInput")
    ident_d = nc.dram_tensor("ident", [128, 128], f32, kind="ExternalInput")
    apart_d = nc.dram_tensor("apart", [128, 1], f32, kind="ExternalInput")
    y_d = nc.dram_tensor("y", [BLKN, 128], f32, kind="ExternalOutput")

    with tile.TileContext(nc) as tc:
        with (
            tc.tile_pool(name="cst", bufs=1) as cst,
            tc.tile_pool(name="sb", bufs=4) as sb,
            tc.tile_pool(name="sp", bufs=10) as sp,
            tc.tile_pool(name="accp", bufs=1) as accp,
            tc.tile_pool(name="hp", bufs=4) as hp,
            tc.tile_pool(name="ps_run", bufs=5, space="PSUM") as ps_run,
            tc.tile_pool(name="ps_epi", bufs=3, space="PSUM") as ps_epi,
            tc.tile_pool(name="dram", bufs=1, space="DRAM") as dram,
        ):
            pools = {"cst": cst, "sb": sb, "sp": sp, "accp": accp, "hp": hp,
                     "ps_run": ps_run, "ps_epi": ps_epi}

            iota_t = cst.tile([128, 128], gdt)
            nc.sync.dma_start(iota_t[:], iota_d[:])
            ident_t = cst.tile([128, 128], f32)
            nc.sync.dma_start(ident_t[:], ident_d[:])
            apart_t = cst.tile([128, 1], f32)
            nc.sync.dma_start(apart_t[:], apart_d[:])
            wself_t = cst.tile([128, BLKN], f32)
            nc.sync.dma_start(wself_t[:], ws_d[:])
            wt = {}
            for name in wshapes:
                wt[name] = cst.tile(list(wshapes[name]), f32, name=f"w_{name}")
                nc.sync.dma_start(wt[name][:], wdr[name][:])
            zero_t = None
            if use_bf16:
                zero_t = cst.tile([128, 128 - H1], gdt)
                nc.vector.memset(zero_t[:], 0.0)

            h1_shard = dram.tile([SLOTS, ge2], gdt)
            h1_full = dram.tile([NROWS2, ge2], gdt, addr_space="Shared")

            def epi1(b, acc_t):
                eps = ps_epi.tile([H1, 128], f32, tag="eps")
                nc.tensor.matmul(eps[:], wt["cW0"][:], acc_t[:], start=True, stop=True)
                h1T = hp.tile([H1, 128], f32, tag="h1T")
                nc.scalar.activation(
                    h1T[:], eps[:], mybir.ActivationFunctionType.Tanh,
                    bias=wt["cb0"][:, 0:1],
                )
                tp = ps_epi.tile([128, H1], f32, tag="eps")
                nc.tensor.transpose(tp[:], h1T[:], ident_t[:H1, :H1])
                h1n = hp.tile([128, H1], gdt, tag="h1n")
                nc.vector.tensor_copy(h1n[:], tp[:])
                nc.scalar.dma_start(
                    h1_shard[b * 128 : (b + 1) * 128, :H1], h1n[:]
                )
                if use_bf16:
                    nc.sync.dma_start(
                        h1_shard[b * 128 : (b + 1) * 128, H1:], zero_t[:]
                    )

            _emit_conv(nc, pools, {
                "T": T1, "feat": D_IN, "gelem": D_IN, "gdt": gdt, "tag": "1",
                "table": lambda be: x_d[BASES[be] : BASES[be] + SIZES1[be], :],
                "self_rows": lambda b: xp_d[b * 128 : (b + 1) * 128, :],
                "idx": idx1_d, "meta": meta1_d,
                "iota_t": iota_t, "apart_t": apart_t, "wself_t": wself_t,
                "epilogue": epi1,
            })

            nc.gpsimd.collective_compute(
                "AllGather",
                mybir.AluOpType.bypass,
                ins=[h1_shard.opt()],
                outs=[h1_full.opt()],
                replica_groups=[list(range(NCORES))],
            )

            def epi2(b, acc_t):
                e1 = ps_epi.tile([H1, 128], f32, tag="eps")
                nc.tensor.matmul(e1[:], wt["cW1"][:], acc_t[:], start=True, stop=True)
                h2T = hp.tile([H1, 128], f32, tag="h2T")
                nc.scalar.activation(
                    h2T[:], e1[:], mybir.ActivationFunctionType.Tanh,
                    bias=wt["cb1"][:, 0:1],
                )
                e2 = ps_epi.tile([H1, 128], f32, tag="eps")
                nc.tensor.matmul(e2[:], wt["fW0"][:], h2T[:], start=True, stop=True)
                h3T = hp.tile([H1, 128], f32, tag="h3T")
                nc.scalar.activation(
                    h3T[:], e2[:], mybir.ActivationFunctionType.Tanh,
                    bias=wt["fb0"][:, 0:1],
                )
                e3 = ps_epi.tile([32, 128], f32, tag="eps")
                nc.tensor.matmul(e3[:], wt["fW1"][:], h3T[:], start=True, stop=True)
                h4T = hp.tile([32, 128], f32, tag="h4T")
                nc.scalar.activation(
                    h4T[:], e3[:], mybir.ActivationFunctionType.Tanh,
                    bias=wt["fb1"][:, 0:1],
                )
                e4 = ps_epi.tile([1, 128], f32, tag="eps")
                nc.tensor.matmul(e4[:], wt["fW2"][:], h4T[:], start=True, stop=True)
                yrow = hp.tile([1, 128], f32, tag="yrow")
                nc.vector.tensor_scalar_add(yrow[:], e4[:], wt["fb2"][0:1, 0:1])
                nc.sync.dma_start(y_d[b : b + 1, :], yrow[:])

            _emit_conv(nc, pools, {
                "T": T2, "feat": H1, "gelem": ge2, "gdt": gdt, "tag": "2",
                "table": lambda be: h1_full[BASES[be] : BASES[be] + SIZES2[be], :],
                "self_rows": lambda b: h1_shard[b * 128 : (b + 1) * 128, :H1],
                "idx": idx2_d, "meta": meta2_d,
                "iota_t": iota_t, "apart_t": apart_t, "wself_t": wself_t,
                "epilogue": epi2,
            })

    nc.compile()
    return nc


# ------------------------------------------------------------------- driver
def kernel(**inputs):
    global LAST_EXEC_TIME_NS, LAST_RESULTS
    use_bf16 = os.environ.get("BASS_GCN_BF16", "1") == "1"
    np_gdt = ml_dtypes.bfloat16 if use_bf16 else np.float32

    x = np.ascontiguousarray(np.asarray(inputs["x"], np.float32))
    ei = np.asarray(inputs["edge_index"], np.int64)
    src, dst = ei[0], ei[1]

    weights = {
        "cW0": np.ascontiguousarray(np.asarray(inputs["cW0"], np.float32)),
        "cb0": np.asarray(inputs["cb0"], np.float32).reshape(H1, 1),
        "cW1": np.ascontiguousarray(np.asarray(inputs["cW1"], np.float32)),
        "cb1": np.asarray(inputs["cb1"], np.float32).reshape(H1, 1),
        "fW0": np.ascontiguousarray(np.asarray(inputs["fW0"], np.float32)),
        "fb0": np.asarray(inputs["fb0"], np.float32).reshape(H1, 1),
        "fW1": np.ascontiguousarray(np.asarray(inputs["fW1"], np.float32)),
        "fb1": np.asarray(inputs["fb1"], np.float32).reshape(32, 1),
        "fW2": np.ascontiguousarray(np.asarray(inputs["fW2"], np.float32)),
        "fb2": np.asarray(inputs["fb2"], np.float32).reshape(1, 1),
    }

    streams, xperms, wselfs, T1, T2, node_of_pos = _plan(src, dst, x)

    nc = _build_program(T1, T2, {k: v.shape for k, v in weights.items()}, use_bf16)

    iota = np.broadcast_to(np.arange(128, dtype=np.float32), (128, 128))
    iota = np.ascontiguousarray(iota.astype(np_gdt))
    ident = np.eye(128, dtype=np.float32)
    apart = np.arange(128, dtype=np.float32).reshape(128, 1)
    xg = np.ascontiguousarray(x.astype(np_gdt))

    in_maps = []
    for c in range(NCORES):
        idx1, meta1, idx2, meta2 = streams[c]
        m = {"xg": xg, "xperm": np.ascontiguousarray(xperms[c].astype(np_gdt)),
             "wself": wselfs[c],
             "idx1": idx1, "meta1": meta1, "idx2": idx2, "meta2": meta2,
             "iota": iota, "ident": ident, "apart": apart}
        m.update(weights)
        in_maps.append(m)

    trace = os.environ.get("BASS_GCN_TRACE") == "1"
    res = run_bass_kernel_spmd(nc, in_maps, list(range(NCORES)), trace=trace)
    if trace:
        LAST_EXEC_TIME_NS = res.exec_time_ns
    LAST_RESULTS = res

    out = np.zeros((N, 1), np.float32)
    for c in range(NCORES):
        yflat = res.results[c]["y"].reshape(SLOTS)
        valid = node_of_pos[c] >= 0
        out[node_of_pos[c][valid], 0] = yflat[valid]
    return out

